# revision 1
# baseline (speedup 1.0000x reference)
"""Trainium2 Bass kernel for the GRUCell-variant problem.

  z = sigmoid(x@Wiz + h@Uhz + bz)
  r = sigmoid(x@Wir + h@Uhr + br)
  n = sigmoid(x@Win + (r*h)@Uhn + bn)
  out = (1-z)*h + z*n

Full shapes: x,h [8192,1024]; W*,U* [1024,1024]; b* [1024].
Sharding: data-parallel over batch across 8 NeuronCores (1024 rows each);
weights replicated; no collectives.

Design (fp16 compute, fp32 PSUM accumulate, zero device transposes):
  - Host pre-transposes x,h to feature-major [D, B_CORE] fp16 and biases
    to [128,8] fp32; weights are cast to fp16 in natural layout. The
    device output is feature-major [D, B_CORE] fp16; the host transposes
    back and upcasts. All layout shuffling is host-side numpy - the
    device does exclusively matmul + sigmoid + elementwise (median rel
    err ~3.5e-4 vs the fp32 reference).
  - Everything SBUF-resident: xT,hT,rh + 6 weight matrices (fp16 halves
    the footprint so it all fits in ~21MiB of SBUF).
  - 768 matmuls of [128d x 128h stationary] @ [128d x 512b moving] fp16
    (1 cyc/row, ~216ns each): ~166us PE floor, and nothing else runs on
    the PE.
  - DMA queues: SP carries inputs + ZN weights + stores (it runs no
    compute, so a long transfer can never block a PSUM drain); ACT
    carries only the R-critical Wir/Uhr chunks + biases, then is free
    for sigmoids.
  - R-phase batch-half-0 pass is o-outer across all 8 hs PSUM groups so
    each arriving 0.25MiB chunk immediately unlocks 8 matmuls; the PE
    stream measures gapless on HW from first matmul to last.
  - 6 warmup matmuls on GpSimd-zeroed scratch ramp the PE DVFS while the
    first chunks are in flight (the real stream then runs at full clock
    from its first instruction); the last hs's n-gate runs as three
    sequential groups (b0, then two b1 halves on disjoint psum column
    ranges) so each drain overlaps the next group's matmuls and the
    post-matmul tail covers only 256 columns, with stores alternated
    across both DMA queues.
Measured: 184.4-186.4us HW exec vs 249.0us for the fp32r baseline.
"""

import sys

if "/opt/trn_rl_repo" not in sys.path:
    sys.path.insert(0, "/opt/trn_rl_repo")

import numpy as np

P = 128
B_FULL = 8192
D = 1024  # d_in == d_h == 1024
N_CORES = 8
B_CORE = B_FULL // N_CORES  # 1024
NS = D // P  # 8 strips of 128 along any 1024 dim
BH = 512  # moving free-dim per matmul (one PSUM bank in fp32)
NBH = B_CORE // BH  # 2 batch halves

_NC_CACHE = {}


def _build_bass():
    import concourse.mybir as mybir
    import concourse.tile as tile
    from concourse import bacc

    F16 = mybir.dt.float16
    F32 = mybir.dt.float32
    SIG = mybir.ActivationFunctionType.Sigmoid

    nc = bacc.Bacc(None, target_bir_lowering=False)

    # x,h arrive pre-transposed to feature-major [D, B] fp16
    xt = nc.dram_tensor("xt", [D, B_CORE], F16, kind="ExternalInput")
    ht = nc.dram_tensor("ht", [D, B_CORE], F16, kind="ExternalInput")
    wts = {
        name: nc.dram_tensor(name, [D, D], F16, kind="ExternalInput")
        for name in ("Wiz", "Uhz", "Wir", "Uhr", "Win", "Uhn")
    }
    bts = {
        name: nc.dram_tensor(name, [P, NS], F32, kind="ExternalInput")
        for name in ("bzt", "brt", "bnt")
    }
    # feature-major output; host transposes back
    out = nc.dram_tensor("out", [D, B_CORE], F16, kind="ExternalOutput")

    with tile.TileContext(nc) as tc:
        with (
            tc.tile_pool(name="big", bufs=1) as big,
            tc.tile_pool(name="dp", bufs=4) as dp,
            tc.tile_pool(name="gt", bufs=8) as gt,
            tc.tile_pool(name="ps", bufs=8, space="PSUM") as psp,
        ):
            # Persistent feature-major activations: [p, o, b] = val[o*128+p, b]
            xT = big.tile([P, NS, B_CORE], F16, tag="xT")
            hT = big.tile([P, NS, B_CORE], F16, tag="hT")
            rh = big.tile([P, NS, B_CORE], F16, tag="rh")
            # Weights natural layout: [p, o, n] = W[o*128+p, n]
            wsb = {}
            for name in ("Wir", "Uhr", "Wiz", "Uhz", "Win", "Uhn"):
                wsb[name] = big.tile(
                    [P, NS, D], F16, tag=f"w_{name}", name=f"w_{name}"
                )
            bias = {}
            for name in ("bzt", "brt", "bnt"):
                bt = big.tile([P, NS], F32, tag=name)
                bias[name] = bt

            # ---- inputs on SP queue: (x,h) chunk pairs per o (R-bh0
            # consumption order), then Wiz/Uhz for the ZN phase.
            for o in range(NS):
                for src, dst in ((xt, xT), (ht, hT)):
                    nc.sync.dma_start(
                        out=dst[:, o, :], in_=src.ap()[o * P:(o + 1) * P, :]
                    )
            # All ZN weights also go on the SP queue: it runs no compute, so
            # long transfers never block a drain. The ACT queue carries only
            # the R-critical Wir/Uhr chunks + biases, then is free for
            # sigmoids.
            for name in ("Wiz", "Uhz", "Win", "Uhn"):
                for c in range(2):
                    o0 = c * (NS // 2)
                    nc.sync.dma_start(
                        out=wsb[name][:, o0:o0 + NS // 2, :],
                        in_=wts[name].ap()[o0 * P:(o0 + NS // 2) * P, :]
                        .rearrange("(o p) n -> p o n", p=P),
                    )
            for o in range(NS):
                for name in ("Wir", "Uhr"):
                    nc.scalar.dma_start(
                        out=wsb[name][:, o, :],
                        in_=wts[name].ap()[o * P:(o + 1) * P, :],
                    )
            for name in ("brt", "bzt", "bnt"):
                nc.scalar.dma_start(out=bias[name], in_=bts[name].ap())

            b0 = slice(0, BH)
            b1 = slice(BH, B_CORE)

            # ---- PE warmup: matmuls on zeroed scratch while the first DMA
            # chunks are still in flight, so the tensor engine's DVFS is
            # already ramped when the real stream begins. The memset runs on
            # GpSimd, whose preamble finishes ~1.5us before Vector's, so the
            # warmup covers the whole 3us ramp window before real data lands.
            scratch = big.tile([P, BH], F16, tag="scratch", name="scratch")
            nc.gpsimd.memset(scratch, 0.0)
            ps_r = [psp.tile([P, BH], F32, tag="mm", name=f"psr{hs}")
                    for hs in range(NS)]
            for _ in range(7):
                nc.tensor.matmul(ps_r[0], scratch[:, 0:P], scratch,
                                 start=True, stop=True)

            # ---- phase R: r = sig(x@Wir + h@Uhr + br); rh = r * hT
            # bh0 pass is o-outer across all 8 hs groups (8 PSUM banks) so
            # each arriving (x,Wir)-chunk o immediately unlocks 8 matmuls --
            # the PE is never blocked behind a group's o=7 chunk while the
            # DMA feed trickles in.
            for o in range(NS):
                for hs in range(NS):
                    nc.tensor.matmul(
                        ps_r[hs], wsb["Wir"][:, o, hs * P:(hs + 1) * P],
                        xT[:, o, b0], start=(o == 0), stop=False,
                    )
                for hs in range(NS):
                    nc.tensor.matmul(
                        ps_r[hs], wsb["Uhr"][:, o, hs * P:(hs + 1) * P],
                        hT[:, o, b0], start=False, stop=(o == NS - 1),
                    )
            for hs in range(NS):
                nc.scalar.activation(
                    rh[:, hs, b0], ps_r[hs], SIG, bias=bias["brt"][:, hs:hs + 1]
                )
                nc.vector.tensor_mul(
                    rh[:, hs, b0], rh[:, hs, b0], hT[:, hs, b0]
                )
            # bh1 pass: everything is resident by now; normal hs-outer groups
            for hs in range(NS):
                cs = slice(hs * P, (hs + 1) * P)
                ps = psp.tile([P, BH], F32, tag="mm")
                for o in range(NS):
                    nc.tensor.matmul(ps, wsb["Wir"][:, o, cs], xT[:, o, b1],
                                     start=(o == 0), stop=False)
                for o in range(NS):
                    nc.tensor.matmul(ps, wsb["Uhr"][:, o, cs], hT[:, o, b1],
                                     start=False, stop=(o == NS - 1))
                nc.scalar.activation(
                    rh[:, hs, b1], ps, SIG, bias=bias["brt"][:, hs:hs + 1]
                )
                nc.vector.tensor_mul(
                    rh[:, hs, b1], rh[:, hs, b1], hT[:, hs, b1]
                )

            # ---- phase ZN + combine
            for hs in range(NS):
                cs = slice(hs * P, (hs + 1) * P)
                ps_z0 = psp.tile([P, BH], F32, tag="mm")
                ps_z1 = psp.tile([P, BH], F32, tag="mm")
                ps_n0 = psp.tile([P, BH], F32, tag="mm")
                ps_n1 = psp.tile([P, BH], F32, tag="mm")
                for o in range(NS):
                    w = wsb["Wiz"][:, o, cs]
                    nc.tensor.matmul(ps_z0, w, xT[:, o, b0],
                                     start=(o == 0), stop=False)
                    nc.tensor.matmul(ps_z1, w, xT[:, o, b1],
                                     start=(o == 0), stop=False)
                for o in range(NS):
                    w = wsb["Uhz"][:, o, cs]
                    nc.tensor.matmul(ps_z0, w, hT[:, o, b0],
                                     start=False, stop=(o == NS - 1))
                    nc.tensor.matmul(ps_z1, w, hT[:, o, b1],
                                     start=False, stop=(o == NS - 1))
                z0 = gt.tile([P, BH], F16, tag="g")
                nc.scalar.activation(z0, ps_z0, SIG, bias=bias["bzt"][:, hs:hs + 1])
                z1 = gt.tile([P, BH], F16, tag="g")
                nc.scalar.activation(z1, ps_z1, SIG, bias=bias["bzt"][:, hs:hs + 1])
                def drain(ps_ap, bs, zt, pq0, nq, qw):
                    """sigmoid + combine + store for a [*, nq*qw] slice of a
                    gate psum; pq0 is the column offset inside the psum/z
                    tiles, bs.start+pq0 the batch offset."""
                    for q in range(nq):
                        p0 = pq0 + q * qw
                        qs = slice(bs.start + p0, bs.start + p0 + qw)
                        pq = slice(p0, p0 + qw)
                        nt = gt.tile([P, qw], F16, tag=f"g{qw}", name="nt")
                        nc.scalar.activation(
                            nt, ps_ap[:, pq], SIG,
                            bias=bias["bnt"][:, hs:hs + 1]
                        )
                        d_t = dp.tile([P, qw], F16, tag=f"d{qw}", name="d_t")
                        nc.vector.tensor_sub(d_t, nt, hT[:, hs, qs])
                        nc.vector.tensor_mul(d_t, d_t, zt[:, pq])
                        nc.vector.tensor_add(d_t, d_t, hT[:, hs, qs])
                        # alternate tail stores across both DMA queues so
                        # they dispatch in parallel
                        eng = nc.scalar if (nq > 1 and q % 2 == 1) else nc.sync
                        eng.dma_start(out=out.ap()[cs, qs], in_=d_t)

                if hs < NS - 1:
                    for o in range(NS):
                        w = wsb["Win"][:, o, cs]
                        nc.tensor.matmul(ps_n0, w, xT[:, o, b0],
                                         start=(o == 0), stop=False)
                        nc.tensor.matmul(ps_n1, w, xT[:, o, b1],
                                         start=(o == 0), stop=False)
                    for o in range(NS):
                        w = wsb["Uhn"][:, o, cs]
                        nc.tensor.matmul(ps_n0, w, rh[:, o, b0],
                                         start=False, stop=(o == NS - 1))
                        nc.tensor.matmul(ps_n1, w, rh[:, o, b1],
                                         start=False, stop=(o == NS - 1))
                    drain(ps_n0, b0, z0, 0, 1, BH)
                    drain(ps_n1, b1, z1, 0, 1, BH)
                else:
                    # last hs: three sequential groups (b0 full, then the
                    # two b1 halves on disjoint psum column ranges) so each
                    # drain overlaps the next group's matmuls and the final
                    # post-matmul tail covers only 256 columns.
                    for o in range(NS):
                        nc.tensor.matmul(ps_n0, wsb["Win"][:, o, cs],
                                         xT[:, o, b0],
                                         start=(o == 0), stop=False)
                    for o in range(NS):
                        nc.tensor.matmul(ps_n0, wsb["Uhn"][:, o, cs],
                                         rh[:, o, b0],
                                         start=False, stop=(o == NS - 1))
                    drain(ps_n0, b0, z0, 0, 2, BH // 2)
                    # each b1 half-group gets its own pool tile (recycling a
                    # long-drained hs-6 bank) so its matmuls never wait on
                    # the previous half's sigmoid drain
                    ps_nh = (ps_n1,
                             psp.tile([P, BH], F32, tag="mm", name="ps_n1b"))
                    for half in range(2):
                        hw_ = BH // 2
                        bq = slice(BH + half * hw_, BH + (half + 1) * hw_)
                        pq = slice(half * hw_, (half + 1) * hw_)
                        for o in range(NS):
                            nc.tensor.matmul(
                                ps_nh[half][:, pq], wsb["Win"][:, o, cs],
                                xT[:, o, bq], start=(o == 0), stop=False,
                            )
                        for o in range(NS):
                            nc.tensor.matmul(
                                ps_nh[half][:, pq], wsb["Uhn"][:, o, cs],
                                rh[:, o, bq],
                                start=False, stop=(o == NS - 1),
                            )
                        drain(ps_nh[half], b1, z1, half * hw_, 2, hw_ // 2)

    nc.compile()
    return nc


def _get_nc():
    if "nc" not in _NC_CACHE:
        _NC_CACHE["nc"] = _build_bass()
    return _NC_CACHE["nc"]


def make_in_maps(inputs):
    f16w = {
        k: np.ascontiguousarray(np.asarray(inputs[k], dtype=np.float16))
        for k in ("Wiz", "Uhz", "Wir", "Uhr", "Win", "Uhn")
    }
    shared = dict(f16w)
    for name, key in (("bzt", "bz"), ("brt", "br"), ("bnt", "bn")):
        shared[name] = np.ascontiguousarray(
            np.asarray(inputs[key], dtype=np.float32).reshape(NS, P).T
        )
    x16 = np.asarray(inputs["x"], dtype=np.float16)
    h16 = np.asarray(inputs["h"], dtype=np.float16)
    in_maps = []
    for c in range(N_CORES):
        sl = slice(c * B_CORE, (c + 1) * B_CORE)
        m = {
            "xt": np.ascontiguousarray(x16[sl].T),
            "ht": np.ascontiguousarray(h16[sl].T),
        }
        m.update(shared)
        in_maps.append(m)
    return in_maps


def kernel(**inputs):
    from concourse.bass_utils import run_bass_kernel_spmd

    nc = _get_nc()
    in_maps = make_in_maps(inputs)
    res = run_bass_kernel_spmd(nc, in_maps, list(range(N_CORES)))
    out = np.concatenate(
        [res.results[c]["out"].T for c in range(N_CORES)], axis=0
    )
    return out.astype(np.float32)



# revision 9
# speedup vs baseline: 1.2740x; 1.2740x over previous
"""Trainium2 Bass kernel for the GRUCell-variant problem.

  z = sigmoid(x@Wiz + h@Uhz + bz)
  r = sigmoid(x@Wir + h@Uhr + br)
  n = sigmoid(x@Win + (r*h)@Uhn + bn)
  out = (1-z)*h + z*n

Full shapes: x,h [8192,1024]; W*,U* [1024,1024]; b* [1024].
Sharding: data-parallel over batch across 8 NeuronCores (1024 rows each);
weights replicated; no collectives.

Mixed-precision design (validated against a CPU bit-sim of TRN fp8):
  - The z-gate multiplies (n-h) ~ O(1) in the final combine, so its
    precision dominates the output error: z GEMMs stay fp16.
  - The r and n gates run as float8e4 (e4m3) DoubleRow matmuls: pairs of
    128-row contraction strips per instruction at 0.5 cyc/row -- 2x PE
    throughput. Weights are host-prescaled by 16 (sigma 0.03 -> 0.5, all
    normals in e4m3); the 1/16 is folded into the sigmoid's scale arg.
  - r is kept in fp16 (rT) and multiplied by the fp16 h copy, quantizing
    the r*h product to fp8 exactly once. The final combine also uses the
    fp16 h. Simulated: l2 rel 7.9e-3, median rel 5.8e-3 (gate 2e-2).
  - Phase order R (fp8, cheap) -> Z (fp16, o-outer across all 8 PSUM
    banks so x16/Wiz stream chunk-by-chunk) -> N (fp8). Three DMA
    queues: SP x8,h8,x16,h16; ACT brt,Wir,Uhr,bz,bn then free for
    sigmoids; GPSIMD scratch-memset then Wiz,Uhz,Win,Uhn (the n-gate
    weights are needed only at N start, ~55us in). 7 warmup matmuls on
    zeroed scratch ramp PE DVFS; last-hs n-gate splits into 3 groups so
    the tail drain covers only 256 columns.
"""

import sys

if "/opt/trn_rl_repo" not in sys.path:
    sys.path.insert(0, "/opt/trn_rl_repo")

import numpy as np
import ml_dtypes

P = 128
B_FULL = 8192
D = 1024  # d_in == d_h == 1024
N_CORES = 8
B_CORE = B_FULL // N_CORES  # 1024
NS = D // P  # 8 strips of 128 along any 1024 dim
NP = NS // 2  # 4 DoubleRow strip-pairs
BH = 512  # moving free-dim per matmul (one PSUM bank in fp32)
WS = 16.0  # fp8 weight prescale (undone in the sigmoid's scale)

E4NP = ml_dtypes.float8_e4m3  # TRN float8e4: max +-240

_NC_CACHE = {}


def _build_bass():
    import concourse.mybir as mybir
    import concourse.tile as tile
    from concourse import bacc

    F16 = mybir.dt.float16
    F32 = mybir.dt.float32
    F8 = mybir.dt.float8e4
    SIG = mybir.ActivationFunctionType.Sigmoid
    DR = mybir.MatmulPerfMode.DoubleRow

    nc = bacc.Bacc(None, target_bir_lowering=False)

    # x,h arrive pre-transposed to feature-major [D, B]; fp16 for the
    # z-gate + elementwise, fp8 for the r/n-gate matmuls.
    xt16 = nc.dram_tensor("xt16", [D, B_CORE], F16, kind="ExternalInput")
    ht16 = nc.dram_tensor("ht16", [D, B_CORE], F16, kind="ExternalInput")
    xt8 = nc.dram_tensor("xt8", [D, B_CORE], F8, kind="ExternalInput")
    ht8 = nc.dram_tensor("ht8", [D, B_CORE], F8, kind="ExternalInput")
    wts16 = {
        name: nc.dram_tensor(name, [D, D], F16, kind="ExternalInput")
        for name in ("Wiz", "Uhz")
    }
    wts8 = {
        name: nc.dram_tensor(name, [D, D], F8, kind="ExternalInput")
        for name in ("Wir", "Uhr", "Win", "Uhn")
    }
    bts = {
        name: nc.dram_tensor(name, [P, NS], F32, kind="ExternalInput")
        for name in ("bzt", "brt", "bnt")
    }
    # feature-major output; host transposes back
    out = nc.dram_tensor("out", [D, B_CORE], F16, kind="ExternalOutput")

    with tile.TileContext(nc) as tc:
        with (
            tc.tile_pool(name="big", bufs=1) as big,
            tc.tile_pool(name="dp", bufs=4) as dp,
            tc.tile_pool(name="gt", bufs=8) as gt,
            tc.tile_pool(name="ps", bufs=8, space="PSUM") as psp,
        ):
            # Persistent feature-major activations: [p, o, b] = val[o*128+p, b]
            xT16 = big.tile([P, NS, B_CORE], F16, tag="xT16")
            hT16 = big.tile([P, NS, B_CORE], F16, tag="hT16")
            xT8 = big.tile([P, NS, B_CORE], F8, tag="xT8")
            hT8 = big.tile([P, NS, B_CORE], F8, tag="hT8")
            rh8 = big.tile([P, NS, B_CORE], F8, tag="rh8")
            rT = big.tile([P, NS, B_CORE], F16, tag="rT")
            zA = big.tile([P, NS, BH], F16, tag="zA")  # z for batch-half 0
            zB = big.tile([P, NS, BH], F16, tag="zB")  # z for batch-half 1
            # Weights natural layout: [p, o, n] = W[o*128+p, n]
            w16 = {
                name: big.tile([P, NS, D], F16, tag=f"w_{name}", name=f"w_{name}")
                for name in ("Wiz", "Uhz")
            }
            w8 = {
                name: big.tile([P, NS, D], F8, tag=f"w_{name}", name=f"w_{name}")
                for name in ("Wir", "Uhr", "Win", "Uhn")
            }
            bias = {
                name: big.tile([P, NS], F32, tag=name, name=name)
                for name in ("bzt", "brt", "bnt")
            }

            # ---- ACT queue: brt first (needed at the R drain), then the
            # R-critical Wir/Uhr strips in consumption order, then bz/bn.
            # After ~2MiB this queue is free for all sigmoids.
            nc.scalar.dma_start(out=bias["brt"], in_=bts["brt"].ap())
            for c in range(2):
                o0 = c * (NS // 2)
                for name in ("Wir", "Uhr"):
                    nc.scalar.dma_start(
                        out=w8[name][:, o0:o0 + NS // 2, :],
                        in_=wts8[name].ap()[o0 * P:(o0 + NS // 2) * P, :]
                        .rearrange("(o p) n -> p o n", p=P),
                    )
            nc.scalar.dma_start(out=bias["bzt"], in_=bts["bzt"].ap())
            nc.scalar.dma_start(out=bias["bnt"], in_=bts["bnt"].ap())

            # ---- SP queue: fp8 x/h halves (R feed, 512KiB transfers),
            # then the fp16 x/h strips in the o-streaming order the Z
            # phase consumes, then the n-gate weights (needed last).
            for c in range(2):
                o0 = c * (NS // 2)
                for src, dst in ((xt8, xT8), (ht8, hT8)):
                    nc.sync.dma_start(
                        out=dst[:, o0:o0 + NS // 2, :],
                        in_=src.ap()[o0 * P:(o0 + NS // 2) * P, :]
                        .rearrange("(o p) n -> p o n", p=P),
                    )
            for src, dst in ((xt16, xT16), (ht16, hT16)):
                for o in range(NS):
                    nc.sync.dma_start(
                        out=dst[:, o, :], in_=src.ap()[o * P:(o + 1) * P, :]
                    )
            for name in ("Win", "Uhn"):
                for c in range(2):
                    o0 = c * (NS // 2)
                    nc.sync.dma_start(
                        out=w8[name][:, o0:o0 + NS // 2, :],
                        in_=wts8[name].ap()[o0 * P:(o0 + NS // 2) * P, :]
                        .rearrange("(o p) n -> p o n", p=P),
                    )

            # ---- GPSIMD queue: warmup scratch memset, then the z-gate
            # fp16 weights in o order (consumed o-outer during Z-bh0).
            scratch = big.tile([P, BH], F16, tag="scratch", name="scratch")
            nc.gpsimd.memset(scratch, 0.0)
            for name in ("Wiz", "Uhz"):
                for o in range(NS):
                    nc.gpsimd.dma_start(
                        out=w16[name][:, o, :],
                        in_=wts16[name].ap()[o * P:(o + 1) * P, :],
                    )

            b0 = slice(0, BH)
            b1 = slice(BH, B_CORE)

            def cs(hs):
                return slice(hs * P, (hs + 1) * P)

            # ---- PE warmup: matmuls on zeroed scratch while the first DMA
            # chunks are in flight, ramping the tensor engine's DVFS far
            # enough that the R stream starts at full clock. A dummy
            # sigmoid on scratch pulls the ACT_TABLE_LOAD (~1.3us) off the
            # critical first-r-sigmoid path.
            ps_r = [psp.tile([P, BH], F32, tag="mm", name=f"psr{hs}")
                    for hs in range(NS)]
            sig_prime = gt.tile([P, NS], F16, tag="gsp", name="sig_prime")
            nc.scalar.activation(sig_prime, scratch[:, 0:NS], SIG,
                                 bias=bias["brt"][:, 0:1], scale=1.0 / WS)
            for _ in range(12):
                nc.tensor.matmul(ps_r[0], scratch[:, 0:P], scratch,
                                 start=True, stop=True)

            # ---- phase R (fp8 DoubleRow): r = sig((x@Wir + h@Uhr)/16 + br)
            # bh0 is strip-pair-outer across all 8 hs PSUM groups so each
            # arriving (x8,Wir) pair chunk immediately unlocks 8 matmuls.
            for op in range(NP):
                pr = slice(2 * op, 2 * op + 2)
                for hs in range(NS):
                    nc.tensor.matmul(
                        ps_r[hs], w8["Wir"][:, pr, cs(hs)], xT8[:, pr, b0],
                        start=(op == 0), stop=False, perf_mode=DR,
                    )
                for hs in range(NS):
                    nc.tensor.matmul(
                        ps_r[hs], w8["Uhr"][:, pr, cs(hs)], hT8[:, pr, b0],
                        start=False, stop=(op == NP - 1), perf_mode=DR,
                    )
            for hs in range(NS):
                nc.scalar.activation(
                    rT[:, hs, b0], ps_r[hs], SIG,
                    bias=bias["brt"][:, hs:hs + 1], scale=1.0 / WS,
                )
            # bh1: everything resident; hs-outer groups
            for hs in range(NS):
                ps = psp.tile([P, BH], F32, tag="mm")
                for op in range(NP):
                    pr = slice(2 * op, 2 * op + 2)
                    nc.tensor.matmul(ps, w8["Wir"][:, pr, cs(hs)],
                                     xT8[:, pr, b1],
                                     start=(op == 0), stop=False, perf_mode=DR)
                for op in range(NP):
                    pr = slice(2 * op, 2 * op + 2)
                    nc.tensor.matmul(ps, w8["Uhr"][:, pr, cs(hs)],
                                     hT8[:, pr, b1],
                                     start=False, stop=(op == NP - 1),
                                     perf_mode=DR)
                nc.scalar.activation(
                    rT[:, hs, b1], ps, SIG,
                    bias=bias["brt"][:, hs:hs + 1], scale=1.0 / WS,
                )
            # rh = r * h16 -> fp8, quantized exactly once. Emitted after all
            # R activations; each mul fires once its hT16 strip has landed
            # (rh is only consumed by the N phase, ~55us later).
            for hs in range(NS):
                for b in (b0, b1):
                    nc.vector.tensor_mul(
                        rh8[:, hs, b], rT[:, hs, b], hT16[:, hs, b]
                    )

            # ---- phase Z (fp16): z = sig(x@Wiz + h@Uhz + bz)
            # bh0 is o-outer across all 8 hs PSUM groups: chunk o of
            # (x16, Wiz) is consumed ~1.7us after chunk o-1, so the fp16
            # feed streams instead of being needed all at once.
            ps_z = [psp.tile([P, BH], F32, tag="mm", name=f"psz{hs}")
                    for hs in range(NS)]
            for o in range(NS):
                for hs in range(NS):
                    nc.tensor.matmul(
                        ps_z[hs], w16["Wiz"][:, o, cs(hs)], xT16[:, o, b0],
                        start=(o == 0), stop=False,
                    )
            for o in range(NS):
                for hs in range(NS):
                    nc.tensor.matmul(
                        ps_z[hs], w16["Uhz"][:, o, cs(hs)], hT16[:, o, b0],
                        start=False, stop=(o == NS - 1),
                    )
            for hs in range(NS):
                nc.scalar.activation(
                    zA[:, hs, :], ps_z[hs], SIG, bias=bias["bzt"][:, hs:hs + 1]
                )
            # bh1: hs-outer
            for hs in range(NS):
                ps = psp.tile([P, BH], F32, tag="mm")
                for o in range(NS):
                    nc.tensor.matmul(ps, w16["Wiz"][:, o, cs(hs)],
                                     xT16[:, o, b1], start=(o == 0), stop=False)
                for o in range(NS):
                    nc.tensor.matmul(ps, w16["Uhz"][:, o, cs(hs)],
                                     hT16[:, o, b1], start=False,
                                     stop=(o == NS - 1))
                nc.scalar.activation(
                    zB[:, hs, :], ps, SIG, bias=bias["bzt"][:, hs:hs + 1]
                )

            # ---- phase N (fp8 DoubleRow) + combine
            def drain(ps_ap, bs, zt, hs, pq0, nq, qw):
                """sigmoid + combine + store for a [*, nq*qw] slice of an
                n-gate psum; pq0 is the column offset inside the psum/z
                tiles, bs.start+pq0 the batch offset."""
                for q in range(nq):
                    p0 = pq0 + q * qw
                    qs = slice(bs.start + p0, bs.start + p0 + qw)
                    pq = slice(p0, p0 + qw)
                    nt = gt.tile([P, qw], F16, tag=f"g{qw}", name="nt")
                    nc.scalar.activation(
                        nt, ps_ap[:, pq], SIG,
                        bias=bias["bnt"][:, hs:hs + 1], scale=1.0 / WS,
                    )
                    d_t = dp.tile([P, qw], F16, tag=f"d{qw}", name="d_t")
                    nc.vector.tensor_sub(d_t, nt, hT16[:, hs, qs])
                    nc.vector.tensor_mul(d_t, d_t, zt[:, pq])
                    nc.vector.tensor_add(d_t, d_t, hT16[:, hs, qs])
                    # alternate tail stores across both DMA queues
                    eng = nc.scalar if (nq > 1 and q % 2 == 1) else nc.sync
                    eng.dma_start(out=out.ap()[cs(hs), qs], in_=d_t)

            for hs in range(NS):
                ps_n0 = psp.tile([P, BH], F32, tag="mm")
                ps_n1 = psp.tile([P, BH], F32, tag="mm")
                if hs < NS - 1:
                    for op in range(NP):
                        pr = slice(2 * op, 2 * op + 2)
                        w = w8["Win"][:, pr, cs(hs)]
                        nc.tensor.matmul(ps_n0, w, xT8[:, pr, b0],
                                         start=(op == 0), stop=False,
                                         perf_mode=DR)
                        nc.tensor.matmul(ps_n1, w, xT8[:, pr, b1],
                                         start=(op == 0), stop=False,
                                         perf_mode=DR)
                    for op in range(NP):
                        pr = slice(2 * op, 2 * op + 2)
                        w = w8["Uhn"][:, pr, cs(hs)]
                        nc.tensor.matmul(ps_n0, w, rh8[:, pr, b0],
                                         start=False, stop=(op == NP - 1),
                                         perf_mode=DR)
                        nc.tensor.matmul(ps_n1, w, rh8[:, pr, b1],
                                         start=False, stop=(op == NP - 1),
                                         perf_mode=DR)
                    drain(ps_n0, b0, zA[:, hs, :], hs, 0, 1, BH)
                    drain(ps_n1, b1, zB[:, hs, :], hs, 0, 1, BH)
                else:
                    # last hs: three sequential groups (b0 full, then the
                    # two b1 halves on disjoint psum column ranges) so each
                    # drain overlaps the next group's matmuls and the final
                    # post-matmul tail covers only 256 columns.
                    for op in range(NP):
                        pr = slice(2 * op, 2 * op + 2)
                        nc.tensor.matmul(ps_n0, w8["Win"][:, pr, cs(hs)],
                                         xT8[:, pr, b0], start=(op == 0),
                                         stop=False, perf_mode=DR)
                    for op in range(NP):
                        pr = slice(2 * op, 2 * op + 2)
                        nc.tensor.matmul(ps_n0, w8["Uhn"][:, pr, cs(hs)],
                                         rh8[:, pr, b0], start=False,
                                         stop=(op == NP - 1), perf_mode=DR)
                    drain(ps_n0, b0, zA[:, hs, :], hs, 0, 2, BH // 2)
                    ps_nh = (ps_n1,
                             psp.tile([P, BH], F32, tag="mm", name="ps_n1b"))
                    for half in range(2):
                        hw_ = BH // 2
                        bq = slice(BH + half * hw_, BH + (half + 1) * hw_)
                        pq = slice(half * hw_, (half + 1) * hw_)
                        for op in range(NP):
                            pr = slice(2 * op, 2 * op + 2)
                            nc.tensor.matmul(
                                ps_nh[half][:, pq], w8["Win"][:, pr, cs(hs)],
                                xT8[:, pr, bq], start=(op == 0), stop=False,
                                perf_mode=DR,
                            )
                        for op in range(NP):
                            pr = slice(2 * op, 2 * op + 2)
                            nc.tensor.matmul(
                                ps_nh[half][:, pq], w8["Uhn"][:, pr, cs(hs)],
                                rh8[:, pr, bq], start=False,
                                stop=(op == NP - 1), perf_mode=DR,
                            )
                        drain(ps_nh[half], b1, zB[:, hs, :], hs,
                              half * hw_, 2, hw_ // 2)

    nc.compile()
    return nc


def _get_nc():
    if "nc" not in _NC_CACHE:
        _NC_CACHE["nc"] = _build_bass()
    return _NC_CACHE["nc"]


def make_in_maps(inputs):
    def q8w(a):
        return np.ascontiguousarray(
            np.clip(np.asarray(a, dtype=np.float32) * WS, -240, 240)
            .astype(E4NP)
        )

    shared = {
        "Wiz": np.ascontiguousarray(np.asarray(inputs["Wiz"], np.float16)),
        "Uhz": np.ascontiguousarray(np.asarray(inputs["Uhz"], np.float16)),
        "Wir": q8w(inputs["Wir"]),
        "Uhr": q8w(inputs["Uhr"]),
        "Win": q8w(inputs["Win"]),
        "Uhn": q8w(inputs["Uhn"]),
    }
    for name, key in (("bzt", "bz"), ("brt", "br"), ("bnt", "bn")):
        shared[name] = np.ascontiguousarray(
            np.asarray(inputs[key], dtype=np.float32).reshape(NS, P).T
        )
    x32 = np.asarray(inputs["x"], np.float32)
    h32 = np.asarray(inputs["h"], np.float32)
    in_maps = []
    for c in range(N_CORES):
        sl = slice(c * B_CORE, (c + 1) * B_CORE)
        xs = np.ascontiguousarray(x32[sl].T)
        hs = np.ascontiguousarray(h32[sl].T)
        m = {
            "xt16": xs.astype(np.float16),
            "ht16": hs.astype(np.float16),
            "xt8": np.clip(xs, -240, 240).astype(E4NP),
            "ht8": np.clip(hs, -240, 240).astype(E4NP),
        }
        m.update(shared)
        in_maps.append(m)
    return in_maps


def kernel(**inputs):
    from concourse.bass_utils import run_bass_kernel_spmd

    nc = _get_nc()
    in_maps = make_in_maps(inputs)
    res = run_bass_kernel_spmd(nc, in_maps, list(range(N_CORES)))
    out = np.concatenate(
        [res.results[c]["out"].T for c in range(N_CORES)], axis=0
    )
    return out.astype(np.float32)
